# revision 24
# baseline (speedup 1.0000x reference)
"""Trainium2 Bass kernel for HGATLinkConv (GNN message passing).

Strategy (8 NeuronCores, SPMD):
  - dst nodes are partitioned contiguously across cores (1250/core); each
    core's edges are those with dst in its range (host-side index prep).
  - Each core computes h = relu((feat @ W) * cj) for ALL nodes (sources can be
    anywhere) via PE matmuls, stores the [N,128] f32 table to DRAM scratch.
  - segment_max: local dst nodes are sorted by in-degree (host).  Round k
    gathers the k-th neighbor's h-row of every node with degree > k (a dense
    prefix of the sorted order), via gpsimd.dma_gather (one 512B row per
    edge), and DVE tensor_max-accumulates into a [128, npos] accumulator
    where position i lives at partition i%128, block i//128 (exactly the
    dma_gather output layout).  Messages are >= 0 and the reference clamps
    the result at 0, so a zero accumulator init + padding with a guaranteed
    zero row is exact.
  - Attention gate (wk = feat @ Wk, per-head L2-normalized q, softmax over
    features) is computed for local nodes only, on ACT/DVE, overlapping the
    gather phase.  Final out = rst * attn.
  - Host un-permutes rows and assembles the full [10000, 128] output.
  - Further refinements: table rows are stored in first-use order so early
    gathers only depend on the first h-store chunks (pruned deps); phase D
    multiplies/stores per 128-position block so early blocks complete while
    the last gathers' DMA drains; transpose evictions are paired into
    [128, 256] PSUM tiles to shorten the DVE copy chain in the head.
    NOTE: SBUF tile addresses follow instruction emission order and the
    gather rate is placement-sensitive (8.62 vs 10.36 us/call); keep the
    const-section load order and gather-tile geometry fixed unless
    re-measuring.
"""

import numpy as np
from contextlib import ExitStack

import concourse.bacc as bacc
import concourse.bass as bass
import concourse.mybir as mybir
import concourse.tile as tile
from concourse.tile_rust import add_dep_helper

F32 = mybir.dt.float32
BF16 = mybir.dt.bfloat16
I16 = mybir.dt.int16
AFT = mybir.ActivationFunctionType
ALU = mybir.AluOpType

# problem constants (hardcoded; kernel.py must be self-contained)
N = 10000
E = 640000
IN_F = 256
OUT_F = 128
HEADS = 8
D_K = 16
TAU = 0.25
NCORES = 8


def _ceil_to(x, m):
    return (x + m - 1) // m * m


def plan(src, dst, n, nloc, ncores, chunk_blocks):
    """Host-side index planning.  Returns per-core permutations, device-layout
    gather index arrays, the global (SPMD-uniform) per-chunk DVE segment
    schedule, and the total block count TB."""
    src = np.asarray(src).astype(np.int64)
    dst = np.asarray(dst).astype(np.int64)
    core_of = dst // nloc
    percore = []
    for c in range(ncores):
        m = core_of == c
        s_c = src[m]
        d_c = dst[m] - c * nloc
        deg = np.bincount(d_c, minlength=nloc)
        perm = np.argsort(-deg, kind="stable")
        sdeg = deg[perm]
        order = np.argsort(d_c, kind="stable")
        s_sorted = s_c[order]
        offs = np.concatenate([[0], np.cumsum(deg)])
        percore.append((perm, sdeg, s_sorted, offs))
    maxdeg = int(max(int(p[1][0]) if len(p[1]) else 0 for p in percore))
    ks = np.arange(maxdeg)
    # n_k per core = number of local nodes with degree > k
    nks = np.stack([(p[1][None, :] > ks[:, None]).sum(1) for p in percore])
    bk = np.maximum(1, -(-nks.max(0) // 128))  # blocks per round, global
    tb0 = int(bk.sum())
    tb = _ceil_to(max(tb0, chunk_blocks), chunk_blocks)
    nchunks = tb // chunk_blocks
    starts = np.concatenate([[0], np.cumsum(bk)])
    segments = [[] for _ in range(nchunks)]
    for k in range(maxdeg):
        gb = int(starts[k])
        b0 = 0
        while b0 < bk[k]:
            chunk, off = divmod(gb, chunk_blocks)
            take = int(min(bk[k] - b0, chunk_blocks - off))
            segments[chunk].append((off, b0, take))
            gb += take
            b0 += take
    zrow = n  # first padded (guaranteed-zero) row of the h table
    flats = []
    for ci_, (perm, sdeg, s_sorted, offs) in enumerate(percore):
        flat = np.full(tb * 128, zrow, np.int64)
        for k in range(maxdeg):
            nk = int(nks[ci_][k])
            if nk == 0:
                continue
            tgt = offs[perm[:nk]] + k
            flat[int(starts[k]) * 128: int(starts[k]) * 128 + nk] = s_sorted[tgt]
        flats.append(flat)
    # table permutation: order node rows by first use across ALL cores (the
    # gather streams are SPMD-synchronous, so use the elementwise-min of the
    # first-use position over cores), so early gathers only need the first
    # h-store chunks.
    first_use = np.full(n + 1, np.iinfo(np.int64).max, np.int64)
    for flat in flats:
        pos = np.arange(tb * 128)
        # first occurrence of each row in this core's stream
        fu = np.full(n + 1, np.iinfo(np.int64).max, np.int64)
        rev = flat[::-1]
        fu[rev] = tb * 128 - 1 - np.arange(tb * 128)
        first_use = np.minimum(first_use, fu)
    order = np.argsort(first_use[:n], kind="stable")
    rho = np.empty(n + 1, np.int64)
    rho[order] = np.arange(n)
    rho[n] = n  # zero row stays at n
    idx_arrs = []
    chunk_maxrow = np.zeros(tb // chunk_blocks, np.int64)
    cb = chunk_blocks * 128
    for flat in flats:
        rflat = rho[flat]
        for c in range(tb // chunk_blocks):
            seg = rflat[c * cb:(c + 1) * cb]
            real = seg[seg < n]
            if len(real):
                chunk_maxrow[c] = max(chunk_maxrow[c], int(real.max()))
        wrapped = rflat.astype(np.int16).reshape(-1, 16).T  # [16, tb*8]
        idx_arrs.append(np.ascontiguousarray(np.tile(wrapped, (8, 1))))
    perms = [p[0] for p in percore]
    return perms, idx_arrs, segments, tb, order, chunk_maxrow


def build(n, in_f, out_f, heads, d_k, tau, nloc, tb, segments, chunk_blocks,
          chunk_maxrow):
    """Build the SPMD Bass program (same structure for every core)."""
    npos = _ceil_to(nloc, 128)
    npad = _ceil_to(n + 1, 1024)
    nchunks = tb // chunk_blocks
    idx_cols = tb * 8
    nmt_l = npos // 128

    nc = bacc.Bacc("TRN2", target_bir_lowering=False, debug=False)
    fcjT_d = nc.dram_tensor("fcjT", [in_f, npad], BF16, kind="ExternalInput")
    featT_l = nc.dram_tensor("featT_l", [in_f, npos], F32, kind="ExternalInput")
    wb_d = nc.dram_tensor("wb", [in_f, out_f], BF16, kind="ExternalInput")
    ident_d = nc.dram_tensor("ident", [128, 128], BF16, kind="ExternalInput")
    wk_d = nc.dram_tensor("wk", [in_f, out_f], F32, kind="ExternalInput")
    ci_d = nc.dram_tensor("ci_sb", [128, nmt_l], F32, kind="ExternalInput")
    idx_d = nc.dram_tensor("idxs", [128, idx_cols], I16, kind="ExternalInput")
    h_d = nc.dram_tensor("h_scratch", [npad, out_f], F32)
    out_d = nc.dram_tensor("out", [128, npos], F32, kind="ExternalOutput")

    with tile.TileContext(nc) as tc, ExitStack() as ctx:
        const = ctx.enter_context(tc.tile_pool(name="const", bufs=1))
        w0b = const.tile([128, out_f], BF16, tag="w0")
        w1b = const.tile([128, out_f], BF16, tag="w1")
        identt = const.tile([128, 128], BF16, tag="ident")
        wk0t = const.tile([128, out_f], F32, tag="wk0")
        wk1t = const.tile([128, out_f], F32, tag="wk1")
        nc.sync.dma_start(w0b[:], wb_d[0:128, :])
        nc.sync.dma_start(w1b[:], wb_d[128:256, :])
        nc.sync.dma_start(identt[:], ident_d[:, :])
        nc.sync.dma_start(wk0t[:], wk_d[0:128, :])
        nc.sync.dma_start(wk1t[:], wk_d[128:256, :])
        cit = const.tile([128, nmt_l], F32, tag="ci")
        nc.sync.dma_start(cit[:], ci_d[:, :])
        idxt = const.tile([128, idx_cols], I16, tag="idx")
        nc.scalar.dma_start(idxt[:], idx_d[:, :])
        fl0 = const.tile([128, npos], F32, tag="fl0")
        fl1 = const.tile([128, npos], F32, tag="fl1")
        nc.scalar.dma_start(fl0[:], featT_l[0:128, :])
        nc.scalar.dma_start(fl1[:], featT_l[128:256, :])
        fj0 = const.tile([128, npad], BF16, tag="fj0")
        fj1 = const.tile([128, npad], BF16, tag="fj1")
        nc.sync.dma_start(fj0[:], fcjT_d[0:128, :])
        nc.sync.dma_start(fj1[:], fcjT_d[128:256, :])
        acc = const.tile([128, npos], F32, tag="acc")
        attn = const.tile([128, npos], F32, tag="attn")
        nc.vector.memset(acc[:], 0.0)

        fpool = ctx.enter_context(tc.tile_pool(name="fpool", bufs=3))
        hpool = ctx.enter_context(tc.tile_pool(name="hpool", bufs=4))
        psa_pool = ctx.enter_context(
            tc.tile_pool(name="psa", bufs=2, space=bass.MemorySpace.PSUM))
        pst_pool = ctx.enter_context(
            tc.tile_pool(name="pst", bufs=3, space=bass.MemorySpace.PSUM))
        pspool = ctx.enter_context(
            tc.tile_pool(name="ps", bufs=2, space=bass.MemorySpace.PSUM))
        apool = ctx.enter_context(tc.tile_pool(name="apool", bufs=2))
        gpool = ctx.enter_context(
            tc.tile_pool(name="gpool", bufs=6 if chunk_blocks <= 16 else 2))

        # ---- phase A: hT = relu(W^T @ (feat*cj)^T) in bf16, then PE
        # transpose to the row-major f32 h table -> DRAM ----
        h_stores = []
        ch_cols = 512
        for c0 in range(0, npad, ch_cols):
            psa = psa_pool.tile([128, ch_cols], F32, tag="psa")
            nc.tensor.matmul(psa[:], w0b[:], fj0[:, c0:c0 + ch_cols],
                             start=True, stop=False)
            nc.tensor.matmul(psa[:], w1b[:], fj1[:, c0:c0 + ch_cols],
                             start=False, stop=True)
            hT = hpool.tile([128, ch_cols], BF16, tag="hT")
            nc.scalar.activation(hT[:], psa[:], AFT.Relu)
            hrow4 = hpool.tile([128, ch_cols], F32, tag="h4")
            for t2 in range(ch_cols // 256):
                pst = pst_pool.tile([128, 2 * out_f], F32, tag="pst")
                for u in range(2):
                    t = t2 * 2 + u
                    nc.tensor.matmul(pst[:, u * out_f:(u + 1) * out_f],
                                     hT[:, t * 128:(t + 1) * 128],
                                     identt[:], start=True, stop=True)
                nc.vector.tensor_scalar_mul(
                    hrow4[:, t2 * 256:(t2 + 1) * 256], pst[:], 1.0)
            dst = h_d[c0:c0 + ch_cols, :].rearrange(
                "(t p) f -> p t f", p=128)
            src3 = hrow4[:].rearrange("p (t f) -> p t f", f=out_f)
            st = nc.sync.dma_start(dst, src3)
            h_stores.append(st)

        # ---- phase B: gather + segment-max ----
        cb8 = chunk_blocks * 8
        gather0 = None
        for chk in range(nchunks):
            # trim trailing pure-padding blocks off the final gather
            nb_real = max(s_[0] + s_[2] for s_ in segments[chk]) \
                if segments[chk] else chunk_blocks
            n_idx = nb_real * 128
            g = gpool.tile([128, chunk_blocks * out_f], F32, tag="g")
            g3 = g[:, :nb_real * out_f].rearrange("p (b e) -> p b e", e=out_f)
            import os
            gi = nc.gpsimd.dma_gather(
                g3, h_d[:, :], idxt[:, chk * cb8:chk * cb8 + nb_real * 8],
                n_idx, n_idx, out_f, elem_step=out_f,
                single_packet=os.environ.get("KQ_SINGLE_PACKET", "1") == "1")
            if gather0 is None:
                gather0 = gi
            need = int(chunk_maxrow[chk]) // 512 + 1
            for st in h_stores[:need]:
                add_dep_helper(gi.ins, st.ins, sync=True,
                               reason="gather reads stored h prefix")
            for gb, ab, nb in segments[chk]:
                nc.vector.tensor_max(
                    acc[:, ab * 128:(ab + nb) * 128],
                    acc[:, ab * 128:(ab + nb) * 128],
                    g[:, gb * out_f:(gb + nb) * out_f])

        # ---- phase C: attention gate for local nodes (overlaps B) ----
        for t in range(nmt_l):
            ps = pspool.tile([128, out_f], F32, tag="ps")
            mm0 = nc.tensor.matmul(ps[:], fl0[:, t * 128:(t + 1) * 128],
                                   wk0t[:], start=True, stop=False)
            if t == 0 and gather0 is not None:
                add_dep_helper(mm0.ins, gather0.ins, sync=True,
                               reason="keep attention out of the head")
            nc.tensor.matmul(ps[:], fl1[:, t * 128:(t + 1) * 128], wk1t[:],
                             start=False, stop=True)
            q = apool.tile([128, out_f], F32, tag="q")
            nc.scalar.activation(q[:], ps[:], AFT.Copy, scale=cit[:, t:t + 1])
            s = apool.tile([128, out_f], F32, tag="s")
            nc.vector.tensor_mul(s[:], q[:], q[:])
            s3 = s[:].rearrange("p (h d) -> p h d", d=d_k)
            hs = apool.tile([128, heads], F32, tag="hs")
            nc.vector.reduce_sum(hs[:], s3, axis=mybir.AxisListType.X)
            hsm = apool.tile([128, heads], F32, tag="hsm")
            nc.vector.tensor_scalar_max(hsm[:], hs[:], 1e-24)
            inv = apool.tile([128, heads], F32, tag="inv")
            nc.vector.reciprocal(inv[:], hsm[:])
            alpha = apool.tile([128, out_f], F32, tag="alpha")
            a3 = alpha[:].rearrange("p (h d) -> p h d", d=d_k)
            nc.vector.tensor_tensor(a3, s3,
                                    inv[:].broadcast_to([128, heads, d_k]),
                                    op=ALU.mult)
            e = apool.tile([128, out_f], F32, tag="e")
            ssum = apool.tile([128, 1], F32, tag="ssum")
            nc.scalar.activation(e[:], alpha[:], AFT.Exp, scale=1.0 / tau,
                                 accum_out=ssum[:])
            sinv = apool.tile([128, 1], F32, tag="sinv")
            nc.vector.reciprocal(sinv[:], ssum[:])
            nc.vector.tensor_scalar_mul(attn[:, t * 128:(t + 1) * 128],
                                        e[:], sinv[:])

        # ---- phase D: out = rst * attn, per 128-position block so early
        # blocks store while the last gathers' DMA is still draining ----
        o = const.tile([128, npos], F32, tag="o")
        for b_ in range(npos // 128):
            sl = slice(b_ * 128, (b_ + 1) * 128)
            nc.vector.tensor_mul(o[:, sl], acc[:, sl], attn[:, sl])
            nc.sync.dma_start(out_d[:, sl], o[:, sl])

    nc.compile()
    return nc


def make_inputs(feat, ci, cj, weight, weight_k, perms, idx_arrs, n, nloc,
                table_order):
    import ml_dtypes
    feat = np.asarray(feat, np.float32)
    ci = np.asarray(ci, np.float32).reshape(-1)
    cj = np.asarray(cj, np.float32).reshape(-1)
    in_f = feat.shape[1]
    npos = _ceil_to(nloc, 128)
    npad = _ceil_to(n + 1, 1024)
    fcjT = np.zeros((in_f, npad), ml_dtypes.bfloat16)
    fcjT[:, :n] = (feat[table_order] * cj[table_order, None]).T.astype(
        ml_dtypes.bfloat16)
    wb = np.ascontiguousarray(
        np.asarray(weight, np.float32).astype(ml_dtypes.bfloat16))
    ident = np.eye(128, dtype=ml_dtypes.bfloat16)
    wk = np.ascontiguousarray(np.asarray(weight_k, np.float32))
    in_maps = []
    for c, (perm, idx_arr) in enumerate(zip(perms, idx_arrs)):
        gids = c * nloc + perm
        fl = np.zeros((in_f, npos), np.float32)
        fl[:, :nloc] = feat[gids].T
        ci_pad = np.zeros(npos, np.float32)
        ci_pad[:nloc] = ci[gids]
        ci_sb = np.ascontiguousarray(ci_pad.reshape(-1, 128).T)
        in_maps.append({
            "fcjT": fcjT, "featT_l": fl, "wb": wb, "ident": ident, "wk": wk,
            "ci_sb": ci_sb, "idxs": idx_arr,
        })
    return in_maps


def decode_outputs(results, perms, n, nloc, out_f):
    npos = _ceil_to(nloc, 128)
    full = np.zeros((n, out_f), np.float32)
    for c, perm in enumerate(perms):
        ob = np.asarray(results[c]["out"])  # [128, npos]
        dec = ob.reshape(128, npos // 128, out_f).transpose(1, 0, 2)
        dec = dec.reshape(npos, out_f)
        full[c * nloc + perm] = dec[:nloc]
    return full


_CACHE = {}

CHUNK_BLOCKS = 7


def run(feat, ci, cj, weight, weight_k, src, dst, *, n=N, ncores=NCORES,
        in_f=IN_F, out_f=OUT_F, heads=HEADS, d_k=D_K, tau=TAU,
        chunk_blocks=CHUNK_BLOCKS, trace=False, tmpdir=None):
    from concourse.bass_utils import run_bass_kernel_spmd
    nloc = n // ncores
    perms, idx_arrs, segments, tb, order, cmr = plan(
        src, dst, n, nloc, ncores, chunk_blocks)
    seg_key = (n, ncores, tb, tuple(int(x) for x in cmr),
               tuple(tuple(s) for ss in segments for s in ss),
               tuple(len(ss) for ss in segments))
    if seg_key in _CACHE:
        nc = _CACHE[seg_key]
    else:
        nc = build(n, in_f, out_f, heads, d_k, tau, nloc, tb, segments,
                   chunk_blocks, cmr)
        _CACHE[seg_key] = nc
    in_maps = make_inputs(feat, ci, cj, weight, weight_k, perms, idx_arrs,
                          n, nloc, order)
    res = run_bass_kernel_spmd(nc, in_maps, core_ids=list(range(ncores)),
                               trace=trace, tmpdir=tmpdir)
    out = decode_outputs(res.results, perms, n, nloc, out_f)
    return out, res


def kernel(feat, ci, cj, weight, weight_k, src, dst):
    out, _ = run(feat, ci, cj, weight, weight_k, src, dst)
    return out



# revision 25
# speedup vs baseline: 1.2018x; 1.2018x over previous
"""Trainium2 Bass kernel for HGATLinkConv (GNN message passing).

Strategy (8 NeuronCores, SPMD):
  - dst nodes are partitioned contiguously across cores (1250/core); each
    core's edges are those with dst in its range (host-side index prep).
  - Each core computes h = relu((feat @ W) * cj) for ALL nodes (sources can be
    anywhere) via PE matmuls, stores the [N,128] f32 table to DRAM scratch.
  - segment_max: local dst nodes are sorted by in-degree (host).  Round k
    gathers the k-th neighbor's h-row of every node with degree > k (a dense
    prefix of the sorted order), via gpsimd.dma_gather (one 512B row per
    edge), and DVE tensor_max-accumulates into a [128, npos] accumulator
    where position i lives at partition i%128, block i//128 (exactly the
    dma_gather output layout).  Messages are >= 0 and the reference clamps
    the result at 0, so a zero accumulator init + padding with a guaranteed
    zero row is exact.
  - Attention gate (wk = feat @ Wk, per-head L2-normalized q, softmax over
    features) is computed for local nodes only, on ACT/DVE, overlapping the
    gather phase.  Final out = rst * attn.
  - Host un-permutes rows and assembles the full [10000, 128] output.
  - Further refinements: table rows are stored in first-use order so early
    gathers only depend on the first h-store chunks (pruned deps); phase D
    multiplies/stores per 128-position block so early blocks complete while
    the last gathers' DMA drains; transpose evictions are paired into
    [128, 256] PSUM tiles to shorten the DVE copy chain in the head.
    NOTE: SBUF tile addresses follow instruction emission order and the
    gather rate is placement-sensitive (8.62 vs 10.36 us/call); keep the
    const-section load order and gather-tile geometry fixed unless
    re-measuring.
"""

import numpy as np
from contextlib import ExitStack

import concourse.bacc as bacc
import concourse.bass as bass
import concourse.mybir as mybir
import concourse.tile as tile
from concourse.tile_rust import add_dep_helper

F32 = mybir.dt.float32
BF16 = mybir.dt.bfloat16
I16 = mybir.dt.int16
AFT = mybir.ActivationFunctionType
ALU = mybir.AluOpType

# problem constants (hardcoded; kernel.py must be self-contained)
N = 10000
E = 640000
IN_F = 256
OUT_F = 128
HEADS = 8
D_K = 16
TAU = 0.25
NCORES = 8


def _ceil_to(x, m):
    return (x + m - 1) // m * m


def plan(src, dst, n, nloc, ncores, chunk_blocks):
    """Host-side index planning.  Returns per-core permutations, device-layout
    gather index arrays, the global (SPMD-uniform) per-chunk DVE segment
    schedule, and the total block count TB."""
    src = np.asarray(src).astype(np.int64)
    dst = np.asarray(dst).astype(np.int64)
    core_of = dst // nloc
    percore = []
    for c in range(ncores):
        m = core_of == c
        s_c = src[m]
        d_c = dst[m] - c * nloc
        deg = np.bincount(d_c, minlength=nloc)
        perm = np.argsort(-deg, kind="stable")
        sdeg = deg[perm]
        order = np.argsort(d_c, kind="stable")
        s_sorted = s_c[order]
        offs = np.concatenate([[0], np.cumsum(deg)])
        percore.append((perm, sdeg, s_sorted, offs))
    maxdeg = int(max(int(p[1][0]) if len(p[1]) else 0 for p in percore))
    ks = np.arange(maxdeg)
    # n_k per core = number of local nodes with degree > k
    nks = np.stack([(p[1][None, :] > ks[:, None]).sum(1) for p in percore])
    bk = np.maximum(1, -(-nks.max(0) // 128))  # blocks per round, global
    tb0 = int(bk.sum())
    tb = _ceil_to(max(tb0, chunk_blocks), chunk_blocks)
    nchunks = tb // chunk_blocks
    starts = np.concatenate([[0], np.cumsum(bk)])
    segments = [[] for _ in range(nchunks)]
    for k in range(maxdeg):
        gb = int(starts[k])
        b0 = 0
        while b0 < bk[k]:
            chunk, off = divmod(gb, chunk_blocks)
            take = int(min(bk[k] - b0, chunk_blocks - off))
            segments[chunk].append((off, b0, take))
            gb += take
            b0 += take
    zrow = n  # first padded (guaranteed-zero) row of the h table
    flats = []
    for ci_, (perm, sdeg, s_sorted, offs) in enumerate(percore):
        flat = np.full(tb * 128, zrow, np.int64)
        for k in range(maxdeg):
            nk = int(nks[ci_][k])
            if nk == 0:
                continue
            tgt = offs[perm[:nk]] + k
            flat[int(starts[k]) * 128: int(starts[k]) * 128 + nk] = s_sorted[tgt]
        flats.append(flat)
    # table permutation: order node rows by first use across ALL cores (the
    # gather streams are SPMD-synchronous, so use the elementwise-min of the
    # first-use position over cores), so early gathers only need the first
    # h-store chunks.
    first_use = np.full(n + 1, np.iinfo(np.int64).max, np.int64)
    for flat in flats:
        pos = np.arange(tb * 128)
        # first occurrence of each row in this core's stream
        fu = np.full(n + 1, np.iinfo(np.int64).max, np.int64)
        rev = flat[::-1]
        fu[rev] = tb * 128 - 1 - np.arange(tb * 128)
        first_use = np.minimum(first_use, fu)
    order = np.argsort(first_use[:n], kind="stable")
    rho = np.empty(n + 1, np.int64)
    rho[order] = np.arange(n)
    rho[n] = n  # zero row stays at n
    idx_arrs = []
    chunk_maxrow = np.zeros(tb // chunk_blocks, np.int64)
    cb = chunk_blocks * 128
    for flat in flats:
        rflat = rho[flat]
        for c in range(tb // chunk_blocks):
            seg = rflat[c * cb:(c + 1) * cb]
            real = seg[seg < n]
            if len(real):
                chunk_maxrow[c] = max(chunk_maxrow[c], int(real.max()))
        wrapped = rflat.astype(np.int16).reshape(-1, 16).T  # [16, tb*8]
        idx_arrs.append(np.ascontiguousarray(np.tile(wrapped, (8, 1))))
    perms = [p[0] for p in percore]
    return perms, idx_arrs, segments, tb, order, chunk_maxrow


def build(n, in_f, out_f, heads, d_k, tau, nloc, tb, segments, chunk_blocks,
          chunk_maxrow):
    """Build the SPMD Bass program (same structure for every core)."""
    npos = _ceil_to(nloc, 128)
    npad = _ceil_to(n + 1, 1024)
    nchunks = tb // chunk_blocks
    idx_cols = tb * 8
    nmt_l = npos // 128

    nc = bacc.Bacc("TRN2", target_bir_lowering=False, debug=False)
    fcjT_d = nc.dram_tensor("fcjT", [in_f, npad], BF16, kind="ExternalInput")
    featT_l = nc.dram_tensor("featT_l", [in_f, npos], F32, kind="ExternalInput")
    wb_d = nc.dram_tensor("wb", [in_f, out_f], BF16, kind="ExternalInput")
    ident_d = nc.dram_tensor("ident", [128, 128], BF16, kind="ExternalInput")
    wk_d = nc.dram_tensor("wk", [in_f, out_f], F32, kind="ExternalInput")
    ci_d = nc.dram_tensor("ci_sb", [128, nmt_l], F32, kind="ExternalInput")
    idx_d = nc.dram_tensor("idxs", [128, idx_cols], I16, kind="ExternalInput")
    h_d = nc.dram_tensor("h_scratch", [npad, out_f], F32)
    out_d = nc.dram_tensor("out", [128, npos], F32, kind="ExternalOutput")

    with tile.TileContext(nc) as tc, ExitStack() as ctx:
        const = ctx.enter_context(tc.tile_pool(name="const", bufs=1))
        w0b = const.tile([128, out_f], BF16, tag="w0")
        w1b = const.tile([128, out_f], BF16, tag="w1")
        identt = const.tile([128, 128], BF16, tag="ident")
        wk0t = const.tile([128, out_f], F32, tag="wk0")
        wk1t = const.tile([128, out_f], F32, tag="wk1")
        nc.sync.dma_start(w0b[:], wb_d[0:128, :])
        nc.sync.dma_start(w1b[:], wb_d[128:256, :])
        nc.sync.dma_start(identt[:], ident_d[:, :])
        nc.sync.dma_start(wk0t[:], wk_d[0:128, :])
        nc.sync.dma_start(wk1t[:], wk_d[128:256, :])
        cit = const.tile([128, nmt_l], F32, tag="ci")
        nc.sync.dma_start(cit[:], ci_d[:, :])
        idxt = const.tile([128, idx_cols], I16, tag="idx")
        nc.scalar.dma_start(idxt[:], idx_d[:, :])
        fl0 = const.tile([128, npos], F32, tag="fl0")
        fl1 = const.tile([128, npos], F32, tag="fl1")
        nc.scalar.dma_start(fl0[:], featT_l[0:128, :])
        nc.scalar.dma_start(fl1[:], featT_l[128:256, :])
        fj0 = const.tile([128, npad], BF16, tag="fj0")
        fj1 = const.tile([128, npad], BF16, tag="fj1")
        nc.sync.dma_start(fj0[:], fcjT_d[0:128, :])
        nc.sync.dma_start(fj1[:], fcjT_d[128:256, :])
        acc = const.tile([128, npos], F32, tag="acc")
        attn = const.tile([128, npos], F32, tag="attn")
        nc.vector.memset(acc[:], 0.0)

        fpool = ctx.enter_context(tc.tile_pool(name="fpool", bufs=3))
        hpool = ctx.enter_context(tc.tile_pool(name="hpool", bufs=4))
        psa_pool = ctx.enter_context(
            tc.tile_pool(name="psa", bufs=2, space=bass.MemorySpace.PSUM))
        pst_pool = ctx.enter_context(
            tc.tile_pool(name="pst", bufs=3, space=bass.MemorySpace.PSUM))
        pspool = ctx.enter_context(
            tc.tile_pool(name="ps", bufs=2, space=bass.MemorySpace.PSUM))
        apool = ctx.enter_context(tc.tile_pool(name="apool", bufs=2))
        gpool = ctx.enter_context(
            tc.tile_pool(name="gpool", bufs=6 if chunk_blocks <= 16 else 2))

        # ---- phase A: hT = relu(W^T @ (feat*cj)^T) in bf16, then PE
        # transpose to the row-major f32 h table -> DRAM ----
        h_stores = []
        ch_cols = 512
        for c0 in range(0, npad, ch_cols):
            psa = psa_pool.tile([128, ch_cols], F32, tag="psa")
            nc.tensor.matmul(psa[:], w0b[:], fj0[:, c0:c0 + ch_cols],
                             start=True, stop=False)
            nc.tensor.matmul(psa[:], w1b[:], fj1[:, c0:c0 + ch_cols],
                             start=False, stop=True)
            hT = hpool.tile([128, ch_cols], BF16, tag="hT")
            nc.scalar.activation(hT[:], psa[:], AFT.Relu)
            hrow4 = hpool.tile([128, ch_cols], F32, tag="h4")
            for t2 in range(ch_cols // 256):
                pst = pst_pool.tile([128, 2 * out_f], F32, tag="pst")
                for u in range(2):
                    t = t2 * 2 + u
                    nc.tensor.matmul(pst[:, u * out_f:(u + 1) * out_f],
                                     hT[:, t * 128:(t + 1) * 128],
                                     identt[:], start=True, stop=True)
                nc.vector.tensor_scalar_mul(
                    hrow4[:, t2 * 256:(t2 + 1) * 256], pst[:], 1.0)
            dst = h_d[c0:c0 + ch_cols, :].rearrange(
                "(t p) f -> p t f", p=128)
            src3 = hrow4[:].rearrange("p (t f) -> p t f", f=out_f)
            st = nc.sync.dma_start(dst, src3)
            h_stores.append(st)

        # ---- phase B: gather + segment-max ----
        cb8 = chunk_blocks * 8
        gather0 = None
        for chk in range(nchunks):
            # trim trailing pure-padding blocks off the final gather
            nb_real = max(s_[0] + s_[2] for s_ in segments[chk]) \
                if segments[chk] else chunk_blocks
            n_idx = nb_real * 128
            g = gpool.tile([128, chunk_blocks * out_f], F32, tag="g")
            g3 = g[:, :nb_real * out_f].rearrange("p (b e) -> p b e", e=out_f)
            import os
            gi = nc.gpsimd.dma_gather(
                g3, h_d[:, :], idxt[:, chk * cb8:chk * cb8 + nb_real * 8],
                n_idx, n_idx, out_f, elem_step=out_f,
                single_packet=os.environ.get("KQ_SINGLE_PACKET", "1") == "1")
            if gather0 is None:
                gather0 = gi
            need = int(chunk_maxrow[chk]) // 512 + 1
            for st in h_stores[:need]:
                add_dep_helper(gi.ins, st.ins, sync=True,
                               reason="gather reads stored h prefix")
            for gb, ab, nb in segments[chk]:
                nc.vector.tensor_max(
                    acc[:, ab * 128:(ab + nb) * 128],
                    acc[:, ab * 128:(ab + nb) * 128],
                    g[:, gb * out_f:(gb + nb) * out_f])

        # ---- phase C: attention gate for local nodes (overlaps B) ----
        for t in range(nmt_l):
            ps = pspool.tile([128, out_f], F32, tag="ps")
            mm0 = nc.tensor.matmul(ps[:], fl0[:, t * 128:(t + 1) * 128],
                                   wk0t[:], start=True, stop=False)
            if t == 0 and gather0 is not None:
                add_dep_helper(mm0.ins, gather0.ins, sync=True,
                               reason="keep attention out of the head")
            nc.tensor.matmul(ps[:], fl1[:, t * 128:(t + 1) * 128], wk1t[:],
                             start=False, stop=True)
            q = apool.tile([128, out_f], F32, tag="q")
            nc.scalar.activation(q[:], ps[:], AFT.Copy, scale=cit[:, t:t + 1])
            s = apool.tile([128, out_f], F32, tag="s")
            nc.vector.tensor_mul(s[:], q[:], q[:])
            s3 = s[:].rearrange("p (h d) -> p h d", d=d_k)
            hs = apool.tile([128, heads], F32, tag="hs")
            nc.vector.reduce_sum(hs[:], s3, axis=mybir.AxisListType.X)
            hsm = apool.tile([128, heads], F32, tag="hsm")
            nc.vector.tensor_scalar_max(hsm[:], hs[:], 1e-24)
            inv = apool.tile([128, heads], F32, tag="inv")
            nc.vector.reciprocal(inv[:], hsm[:])
            alpha = apool.tile([128, out_f], F32, tag="alpha")
            a3 = alpha[:].rearrange("p (h d) -> p h d", d=d_k)
            nc.vector.tensor_tensor(a3, s3,
                                    inv[:].broadcast_to([128, heads, d_k]),
                                    op=ALU.mult)
            e = apool.tile([128, out_f], F32, tag="e")
            ssum = apool.tile([128, 1], F32, tag="ssum")
            nc.scalar.activation(e[:], alpha[:], AFT.Exp, scale=1.0 / tau,
                                 accum_out=ssum[:])
            sinv = apool.tile([128, 1], F32, tag="sinv")
            nc.vector.reciprocal(sinv[:], ssum[:])
            nc.vector.tensor_scalar_mul(attn[:, t * 128:(t + 1) * 128],
                                        e[:], sinv[:])

        # ---- phase D: out = rst * attn, per 128-position block so early
        # blocks store while the last gathers' DMA is still draining ----
        o = const.tile([128, npos], F32, tag="o")
        for b_ in range(npos // 128):
            sl = slice(b_ * 128, (b_ + 1) * 128)
            nc.vector.tensor_mul(o[:, sl], acc[:, sl], attn[:, sl])
            nc.sync.dma_start(out_d[:, sl], o[:, sl])

    nc.compile()
    return nc


def make_inputs(feat, ci, cj, weight, weight_k, perms, idx_arrs, n, nloc,
                table_order):
    import ml_dtypes
    feat = np.asarray(feat, np.float32)
    ci = np.asarray(ci, np.float32).reshape(-1)
    cj = np.asarray(cj, np.float32).reshape(-1)
    in_f = feat.shape[1]
    npos = _ceil_to(nloc, 128)
    npad = _ceil_to(n + 1, 1024)
    fcjT = np.zeros((in_f, npad), ml_dtypes.bfloat16)
    fcjT[:, :n] = (feat[table_order] * cj[table_order, None]).T.astype(
        ml_dtypes.bfloat16)
    wb = np.ascontiguousarray(
        np.asarray(weight, np.float32).astype(ml_dtypes.bfloat16))
    ident = np.eye(128, dtype=ml_dtypes.bfloat16)
    wk = np.ascontiguousarray(np.asarray(weight_k, np.float32))
    in_maps = []
    for c, (perm, idx_arr) in enumerate(zip(perms, idx_arrs)):
        gids = c * nloc + perm
        fl = np.zeros((in_f, npos), np.float32)
        fl[:, :nloc] = feat[gids].T
        ci_pad = np.zeros(npos, np.float32)
        ci_pad[:nloc] = ci[gids]
        ci_sb = np.ascontiguousarray(ci_pad.reshape(-1, 128).T)
        in_maps.append({
            "fcjT": fcjT, "featT_l": fl, "wb": wb, "ident": ident, "wk": wk,
            "ci_sb": ci_sb, "idxs": idx_arr,
        })
    return in_maps


def decode_outputs(results, perms, n, nloc, out_f):
    npos = _ceil_to(nloc, 128)
    full = np.zeros((n, out_f), np.float32)
    for c, perm in enumerate(perms):
        ob = np.asarray(results[c]["out"])  # [128, npos]
        dec = ob.reshape(128, npos // 128, out_f).transpose(1, 0, 2)
        dec = dec.reshape(npos, out_f)
        full[c * nloc + perm] = dec[:nloc]
    return full


_CACHE = {}

CHUNK_BLOCKS = 8


def run(feat, ci, cj, weight, weight_k, src, dst, *, n=N, ncores=NCORES,
        in_f=IN_F, out_f=OUT_F, heads=HEADS, d_k=D_K, tau=TAU,
        chunk_blocks=CHUNK_BLOCKS, trace=False, tmpdir=None):
    from concourse.bass_utils import run_bass_kernel_spmd
    nloc = n // ncores
    perms, idx_arrs, segments, tb, order, cmr = plan(
        src, dst, n, nloc, ncores, chunk_blocks)
    seg_key = (n, ncores, tb, tuple(int(x) for x in cmr),
               tuple(tuple(s) for ss in segments for s in ss),
               tuple(len(ss) for ss in segments))
    if seg_key in _CACHE:
        nc = _CACHE[seg_key]
    else:
        nc = build(n, in_f, out_f, heads, d_k, tau, nloc, tb, segments,
                   chunk_blocks, cmr)
        _CACHE[seg_key] = nc
    in_maps = make_inputs(feat, ci, cj, weight, weight_k, perms, idx_arrs,
                          n, nloc, order)
    res = run_bass_kernel_spmd(nc, in_maps, core_ids=list(range(ncores)),
                               trace=trace, tmpdir=tmpdir)
    out = decode_outputs(res.results, perms, n, nloc, out_f)
    return out, res


def kernel(feat, ci, cj, weight, weight_k, src, dst):
    out, _ = run(feat, ci, cj, weight, weight_k, src, dst)
    return out



# revision 26
# speedup vs baseline: 1.2075x; 1.0048x over previous
"""Trainium2 Bass kernel for HGATLinkConv (GNN message passing).

Strategy (8 NeuronCores, SPMD):
  - dst nodes are partitioned contiguously across cores (1250/core); each
    core's edges are those with dst in its range (host-side index prep).
  - Each core computes h = relu((feat @ W) * cj) for ALL nodes (sources can be
    anywhere) via PE matmuls, stores the [N,128] f32 table to DRAM scratch.
  - segment_max: local dst nodes are sorted by in-degree (host).  Round k
    gathers the k-th neighbor's h-row of every node with degree > k (a dense
    prefix of the sorted order), via gpsimd.dma_gather (one 512B row per
    edge), and DVE tensor_max-accumulates into a [128, npos] accumulator
    where position i lives at partition i%128, block i//128 (exactly the
    dma_gather output layout).  Messages are >= 0 and the reference clamps
    the result at 0, so a zero accumulator init + padding with a guaranteed
    zero row is exact.
  - Attention gate (wk = feat @ Wk, per-head L2-normalized q, softmax over
    features) is computed for local nodes only, on ACT/DVE, overlapping the
    gather phase.  Final out = rst * attn.
  - Host un-permutes rows and assembles the full [10000, 128] output.
  - Further refinements: table rows are stored in first-use order so early
    gathers only depend on the first h-store chunks (pruned deps); phase D
    multiplies/stores per 128-position block so early blocks complete while
    the last gathers' DMA drains; transpose evictions are paired into
    [128, 256] PSUM tiles to shorten the DVE copy chain in the head.
    NOTE: SBUF tile addresses follow instruction emission order and the
    gather rate is placement-sensitive (8.62 vs 10.36 us/call); keep the
    const-section load order and gather-tile geometry fixed unless
    re-measuring.
"""

import numpy as np
from contextlib import ExitStack

import concourse.bacc as bacc
import concourse.bass as bass
import concourse.mybir as mybir
import concourse.tile as tile
from concourse.tile_rust import add_dep_helper

F32 = mybir.dt.float32
BF16 = mybir.dt.bfloat16
I16 = mybir.dt.int16
AFT = mybir.ActivationFunctionType
ALU = mybir.AluOpType

# problem constants (hardcoded; kernel.py must be self-contained)
N = 10000
E = 640000
IN_F = 256
OUT_F = 128
HEADS = 8
D_K = 16
TAU = 0.25
NCORES = 8


def _ceil_to(x, m):
    return (x + m - 1) // m * m


def plan(src, dst, n, nloc, ncores, chunk_blocks):
    """Host-side index planning.  Returns per-core permutations, device-layout
    gather index arrays, the global (SPMD-uniform) per-chunk DVE segment
    schedule, and the total block count TB."""
    src = np.asarray(src).astype(np.int64)
    dst = np.asarray(dst).astype(np.int64)
    core_of = dst // nloc
    percore = []
    for c in range(ncores):
        m = core_of == c
        s_c = src[m]
        d_c = dst[m] - c * nloc
        deg = np.bincount(d_c, minlength=nloc)
        perm = np.argsort(-deg, kind="stable")
        sdeg = deg[perm]
        order = np.argsort(d_c, kind="stable")
        s_sorted = s_c[order]
        offs = np.concatenate([[0], np.cumsum(deg)])
        percore.append((perm, sdeg, s_sorted, offs))
    maxdeg = int(max(int(p[1][0]) if len(p[1]) else 0 for p in percore))
    ks = np.arange(maxdeg)
    # n_k per core = number of local nodes with degree > k
    nks = np.stack([(p[1][None, :] > ks[:, None]).sum(1) for p in percore])
    bk = np.maximum(1, -(-nks.max(0) // 128))  # blocks per round, global
    tb0 = int(bk.sum())
    tb = _ceil_to(max(tb0, chunk_blocks), chunk_blocks)
    nchunks = tb // chunk_blocks
    starts = np.concatenate([[0], np.cumsum(bk)])
    segments = [[] for _ in range(nchunks)]
    for k in range(maxdeg):
        gb = int(starts[k])
        b0 = 0
        while b0 < bk[k]:
            chunk, off = divmod(gb, chunk_blocks)
            take = int(min(bk[k] - b0, chunk_blocks - off))
            segments[chunk].append((off, b0, take))
            gb += take
            b0 += take
    zrow = n  # first padded (guaranteed-zero) row of the h table
    flats = []
    for ci_, (perm, sdeg, s_sorted, offs) in enumerate(percore):
        flat = np.full(tb * 128, zrow, np.int64)
        for k in range(maxdeg):
            nk = int(nks[ci_][k])
            if nk == 0:
                continue
            tgt = offs[perm[:nk]] + k
            flat[int(starts[k]) * 128: int(starts[k]) * 128 + nk] = s_sorted[tgt]
        flats.append(flat)
    # PER-CORE table permutation: each core orders its own h-table rows by
    # first use in its own gather stream (fcjT is a per-core input, so the
    # table content may differ per core); only chunk_maxrow -- the store
    # prefix each gather chunk depends on -- must be the SPMD max.  This
    # keeps gather 0's dependency at ~2 store chunks instead of ~16.
    idx_arrs = []
    orders = []
    chunk_maxrow = np.zeros(tb // chunk_blocks, np.int64)
    cb = chunk_blocks * 128
    for flat in flats:
        fu = np.full(n + 1, np.iinfo(np.int64).max, np.int64)
        rev = flat[::-1]
        fu[rev] = tb * 128 - 1 - np.arange(tb * 128)
        order_c = np.argsort(fu[:n], kind="stable")
        rho = np.empty(n + 1, np.int64)
        rho[order_c] = np.arange(n)
        rho[n] = n  # zero row stays at n
        rflat = rho[flat]
        for c in range(tb // chunk_blocks):
            seg = rflat[c * cb:(c + 1) * cb]
            real = seg[seg < n]
            if len(real):
                chunk_maxrow[c] = max(chunk_maxrow[c], int(real.max()))
        wrapped = rflat.astype(np.int16).reshape(-1, 16).T  # [16, tb*8]
        idx_arrs.append(np.ascontiguousarray(np.tile(wrapped, (8, 1))))
        orders.append(order_c)
    perms = [p[0] for p in percore]
    return perms, idx_arrs, segments, tb, orders, chunk_maxrow


def build(n, in_f, out_f, heads, d_k, tau, nloc, tb, segments, chunk_blocks,
          chunk_maxrow):
    """Build the SPMD Bass program (same structure for every core)."""
    npos = _ceil_to(nloc, 128)
    npad = _ceil_to(n + 1, 1024)
    nchunks = tb // chunk_blocks
    idx_cols = tb * 8
    nmt_l = npos // 128

    nc = bacc.Bacc("TRN2", target_bir_lowering=False, debug=False)
    fcjT_d = nc.dram_tensor("fcjT", [in_f, npad], BF16, kind="ExternalInput")
    featT_l = nc.dram_tensor("featT_l", [in_f, npos], F32, kind="ExternalInput")
    wb_d = nc.dram_tensor("wb", [in_f, out_f], BF16, kind="ExternalInput")
    ident_d = nc.dram_tensor("ident", [128, 128], BF16, kind="ExternalInput")
    wk_d = nc.dram_tensor("wk", [in_f, out_f], F32, kind="ExternalInput")
    ci_d = nc.dram_tensor("ci_sb", [128, nmt_l], F32, kind="ExternalInput")
    idx_d = nc.dram_tensor("idxs", [128, idx_cols], I16, kind="ExternalInput")
    h_d = nc.dram_tensor("h_scratch", [npad, out_f], F32)
    out_d = nc.dram_tensor("out", [128, npos], F32, kind="ExternalOutput")

    with tile.TileContext(nc) as tc, ExitStack() as ctx:
        const = ctx.enter_context(tc.tile_pool(name="const", bufs=1))
        w0b = const.tile([128, out_f], BF16, tag="w0")
        w1b = const.tile([128, out_f], BF16, tag="w1")
        identt = const.tile([128, 128], BF16, tag="ident")
        wk0t = const.tile([128, out_f], F32, tag="wk0")
        wk1t = const.tile([128, out_f], F32, tag="wk1")
        nc.sync.dma_start(w0b[:], wb_d[0:128, :])
        nc.sync.dma_start(w1b[:], wb_d[128:256, :])
        nc.sync.dma_start(identt[:], ident_d[:, :])
        nc.sync.dma_start(wk0t[:], wk_d[0:128, :])
        nc.sync.dma_start(wk1t[:], wk_d[128:256, :])
        cit = const.tile([128, nmt_l], F32, tag="ci")
        nc.sync.dma_start(cit[:], ci_d[:, :])
        idxt = const.tile([128, idx_cols], I16, tag="idx")
        nc.scalar.dma_start(idxt[:], idx_d[:, :])
        fl0 = const.tile([128, npos], F32, tag="fl0")
        fl1 = const.tile([128, npos], F32, tag="fl1")
        nc.scalar.dma_start(fl0[:], featT_l[0:128, :])
        nc.scalar.dma_start(fl1[:], featT_l[128:256, :])
        fj0 = const.tile([128, npad], BF16, tag="fj0")
        fj1 = const.tile([128, npad], BF16, tag="fj1")
        nc.sync.dma_start(fj0[:], fcjT_d[0:128, :])
        nc.sync.dma_start(fj1[:], fcjT_d[128:256, :])
        acc = const.tile([128, npos], F32, tag="acc")
        attn = const.tile([128, npos], F32, tag="attn")
        nc.vector.memset(acc[:], 0.0)

        fpool = ctx.enter_context(tc.tile_pool(name="fpool", bufs=3))
        hpool = ctx.enter_context(tc.tile_pool(name="hpool", bufs=4))
        psa_pool = ctx.enter_context(
            tc.tile_pool(name="psa", bufs=2, space=bass.MemorySpace.PSUM))
        pst_pool = ctx.enter_context(
            tc.tile_pool(name="pst", bufs=3, space=bass.MemorySpace.PSUM))
        pspool = ctx.enter_context(
            tc.tile_pool(name="ps", bufs=2, space=bass.MemorySpace.PSUM))
        apool = ctx.enter_context(tc.tile_pool(name="apool", bufs=2))
        gpool = ctx.enter_context(
            tc.tile_pool(name="gpool", bufs=6 if chunk_blocks <= 16 else 2))

        # ---- phase A: hT = relu(W^T @ (feat*cj)^T) in bf16, then PE
        # transpose to the row-major f32 h table -> DRAM ----
        h_stores = []
        ch_cols = 512
        for c0 in range(0, npad, ch_cols):
            psa = psa_pool.tile([128, ch_cols], F32, tag="psa")
            nc.tensor.matmul(psa[:], w0b[:], fj0[:, c0:c0 + ch_cols],
                             start=True, stop=False)
            nc.tensor.matmul(psa[:], w1b[:], fj1[:, c0:c0 + ch_cols],
                             start=False, stop=True)
            hT = hpool.tile([128, ch_cols], BF16, tag="hT")
            nc.scalar.activation(hT[:], psa[:], AFT.Relu)
            hrow4 = hpool.tile([128, ch_cols], F32, tag="h4")
            for t2 in range(ch_cols // 256):
                pst = pst_pool.tile([128, 2 * out_f], F32, tag="pst")
                for u in range(2):
                    t = t2 * 2 + u
                    nc.tensor.matmul(pst[:, u * out_f:(u + 1) * out_f],
                                     hT[:, t * 128:(t + 1) * 128],
                                     identt[:], start=True, stop=True)
                nc.vector.tensor_scalar_mul(
                    hrow4[:, t2 * 256:(t2 + 1) * 256], pst[:], 1.0)
            dst = h_d[c0:c0 + ch_cols, :].rearrange(
                "(t p) f -> p t f", p=128)
            src3 = hrow4[:].rearrange("p (t f) -> p t f", f=out_f)
            st = nc.sync.dma_start(dst, src3)
            h_stores.append(st)

        # ---- phase B: gather + segment-max ----
        cb8 = chunk_blocks * 8
        gather0 = None
        for chk in range(nchunks):
            # trim trailing pure-padding blocks off the final gather
            nb_real = max(s_[0] + s_[2] for s_ in segments[chk]) \
                if segments[chk] else chunk_blocks
            n_idx = nb_real * 128
            g = gpool.tile([128, chunk_blocks * out_f], F32, tag="g")
            g3 = g[:, :nb_real * out_f].rearrange("p (b e) -> p b e", e=out_f)
            import os
            gi = nc.gpsimd.dma_gather(
                g3, h_d[:, :], idxt[:, chk * cb8:chk * cb8 + nb_real * 8],
                n_idx, n_idx, out_f, elem_step=out_f,
                single_packet=os.environ.get("KQ_SINGLE_PACKET", "1") == "1")
            if gather0 is None:
                gather0 = gi
            need = int(chunk_maxrow[chk]) // 512 + 1
            for st in h_stores[:need]:
                add_dep_helper(gi.ins, st.ins, sync=True,
                               reason="gather reads stored h prefix")
            for gb, ab, nb in segments[chk]:
                nc.vector.tensor_max(
                    acc[:, ab * 128:(ab + nb) * 128],
                    acc[:, ab * 128:(ab + nb) * 128],
                    g[:, gb * out_f:(gb + nb) * out_f])

        # ---- phase C: attention gate for local nodes (overlaps B) ----
        for t in range(nmt_l):
            ps = pspool.tile([128, out_f], F32, tag="ps")
            mm0 = nc.tensor.matmul(ps[:], fl0[:, t * 128:(t + 1) * 128],
                                   wk0t[:], start=True, stop=False)
            if t == 0 and gather0 is not None:
                add_dep_helper(mm0.ins, gather0.ins, sync=True,
                               reason="keep attention out of the head")
            nc.tensor.matmul(ps[:], fl1[:, t * 128:(t + 1) * 128], wk1t[:],
                             start=False, stop=True)
            q = apool.tile([128, out_f], F32, tag="q")
            nc.scalar.activation(q[:], ps[:], AFT.Copy, scale=cit[:, t:t + 1])
            s = apool.tile([128, out_f], F32, tag="s")
            nc.vector.tensor_mul(s[:], q[:], q[:])
            s3 = s[:].rearrange("p (h d) -> p h d", d=d_k)
            hs = apool.tile([128, heads], F32, tag="hs")
            nc.vector.reduce_sum(hs[:], s3, axis=mybir.AxisListType.X)
            hsm = apool.tile([128, heads], F32, tag="hsm")
            nc.vector.tensor_scalar_max(hsm[:], hs[:], 1e-24)
            inv = apool.tile([128, heads], F32, tag="inv")
            nc.vector.reciprocal(inv[:], hsm[:])
            alpha = apool.tile([128, out_f], F32, tag="alpha")
            a3 = alpha[:].rearrange("p (h d) -> p h d", d=d_k)
            nc.vector.tensor_tensor(a3, s3,
                                    inv[:].broadcast_to([128, heads, d_k]),
                                    op=ALU.mult)
            e = apool.tile([128, out_f], F32, tag="e")
            ssum = apool.tile([128, 1], F32, tag="ssum")
            nc.scalar.activation(e[:], alpha[:], AFT.Exp, scale=1.0 / tau,
                                 accum_out=ssum[:])
            sinv = apool.tile([128, 1], F32, tag="sinv")
            nc.vector.reciprocal(sinv[:], ssum[:])
            nc.vector.tensor_scalar_mul(attn[:, t * 128:(t + 1) * 128],
                                        e[:], sinv[:])

        # ---- phase D: out = rst * attn, per 128-position block so early
        # blocks store while the last gathers' DMA is still draining ----
        o = const.tile([128, npos], F32, tag="o")
        for b_ in range(npos // 128):
            sl = slice(b_ * 128, (b_ + 1) * 128)
            nc.vector.tensor_mul(o[:, sl], acc[:, sl], attn[:, sl])
            nc.sync.dma_start(out_d[:, sl], o[:, sl])

    nc.compile()
    return nc


def make_inputs(feat, ci, cj, weight, weight_k, perms, idx_arrs, n, nloc,
                table_orders):
    import ml_dtypes
    feat = np.asarray(feat, np.float32)
    ci = np.asarray(ci, np.float32).reshape(-1)
    cj = np.asarray(cj, np.float32).reshape(-1)
    in_f = feat.shape[1]
    npos = _ceil_to(nloc, 128)
    npad = _ceil_to(n + 1, 1024)
    wb = np.ascontiguousarray(
        np.asarray(weight, np.float32).astype(ml_dtypes.bfloat16))
    ident = np.eye(128, dtype=ml_dtypes.bfloat16)
    wk = np.ascontiguousarray(np.asarray(weight_k, np.float32))
    in_maps = []
    for c, (perm, idx_arr) in enumerate(zip(perms, idx_arrs)):
        to = table_orders[c]
        fcjT = np.zeros((in_f, npad), ml_dtypes.bfloat16)
        fcjT[:, :n] = (feat[to] * cj[to, None]).T.astype(ml_dtypes.bfloat16)
        gids = c * nloc + perm
        fl = np.zeros((in_f, npos), np.float32)
        fl[:, :nloc] = feat[gids].T
        ci_pad = np.zeros(npos, np.float32)
        ci_pad[:nloc] = ci[gids]
        ci_sb = np.ascontiguousarray(ci_pad.reshape(-1, 128).T)
        in_maps.append({
            "fcjT": fcjT, "featT_l": fl, "wb": wb, "ident": ident, "wk": wk,
            "ci_sb": ci_sb, "idxs": idx_arr,
        })
    return in_maps


def decode_outputs(results, perms, n, nloc, out_f):
    npos = _ceil_to(nloc, 128)
    full = np.zeros((n, out_f), np.float32)
    for c, perm in enumerate(perms):
        ob = np.asarray(results[c]["out"])  # [128, npos]
        dec = ob.reshape(128, npos // 128, out_f).transpose(1, 0, 2)
        dec = dec.reshape(npos, out_f)
        full[c * nloc + perm] = dec[:nloc]
    return full


_CACHE = {}

CHUNK_BLOCKS = 8


def run(feat, ci, cj, weight, weight_k, src, dst, *, n=N, ncores=NCORES,
        in_f=IN_F, out_f=OUT_F, heads=HEADS, d_k=D_K, tau=TAU,
        chunk_blocks=CHUNK_BLOCKS, trace=False, tmpdir=None):
    from concourse.bass_utils import run_bass_kernel_spmd
    nloc = n // ncores
    perms, idx_arrs, segments, tb, orders, cmr = plan(
        src, dst, n, nloc, ncores, chunk_blocks)
    seg_key = (n, ncores, tb, tuple(int(x) for x in cmr),
               tuple(tuple(s) for ss in segments for s in ss),
               tuple(len(ss) for ss in segments))
    if seg_key in _CACHE:
        nc = _CACHE[seg_key]
    else:
        nc = build(n, in_f, out_f, heads, d_k, tau, nloc, tb, segments,
                   chunk_blocks, cmr)
        _CACHE[seg_key] = nc
    in_maps = make_inputs(feat, ci, cj, weight, weight_k, perms, idx_arrs,
                          n, nloc, orders)
    res = run_bass_kernel_spmd(nc, in_maps, core_ids=list(range(ncores)),
                               trace=trace, tmpdir=tmpdir)
    out = decode_outputs(res.results, perms, n, nloc, out_f)
    return out, res


def kernel(feat, ci, cj, weight, weight_k, src, dst):
    out, _ = run(feat, ci, cj, weight, weight_k, src, dst)
    return out

